# revision 1
# baseline (speedup 1.0000x reference)
"""Trainium2 Bass kernel for nn_LoRAPool (MoE top-2 LoRA expert pool).

Math (reference):
    gates[t,e] = p_L[t,e] if e in top-2 of p_L[t,:] else 0
    hr[t,e,r]  = sum_d h[t,d] * A[e,r,d]
    out[t,d]   = sum_{e,r} hr[t,e,r] * 2.0 * gates[t,e] * B[e,d,r]

Folded into two dense matmuls over c = (e,r) in [0,128):
    A_cat[d,c] = 2.0 * A[e,r,d];  B_cat[c,d] = B[e,d,r]
    U^T[c,t]   = sum_d A_cat[d,c] h[t,d]        (stage 1, PE)
    Us[c,t]    = U^T[c,t] * gates[t, c//16]     (gating, DVE)
    out[t,d]   = sum_c Us[c,t] B_cat[c,d]       (stage 2, PE)

Sharding: tokens (4*4096 = 16384) split evenly across 8 cores; A/B and
small helper matrices are replicated.
"""

import numpy as np

N_CORES = 8
B_SZ, S_SZ, D = 4, 4096, 2048
E, R, C = 8, 16, 128
T_FULL = B_SZ * S_SZ            # 16384 tokens
T_CORE = T_FULL // N_CORES      # 2048 tokens per core
GROUP = 512                     # token group (matmul moving dim)
N_GROUPS = T_CORE // GROUP      # 4
N_SUB = GROUP // 128            # 4 sub-tiles of 128 tokens
KD = D // 128                   # 16 contraction chunks
SCALING = 2.0

_CACHE = {}


def _build_nc(use_f32r=True, split_waits=True):
    import concourse.bass as bass
    import concourse.tile as tile
    import concourse.mybir as mybir
    from contextlib import ExitStack

    f32 = mybir.dt.float32
    mm_dt = mybir.dt.float32r if use_f32r else f32

    nc = bass.Bass()
    h_d = nc.declare_dram_parameter("h", [T_CORE, D], f32, isOutput=False)
    p_d = nc.declare_dram_parameter("p_L", [T_CORE, E], f32, isOutput=False)
    a_d = nc.declare_dram_parameter("A_cat", [D, C], f32, isOutput=False)
    b_d = nc.declare_dram_parameter("B_cat", [C, D], f32, isOutput=False)
    m_d = nc.declare_dram_parameter("Mexp", [E, C], f32, isOutput=False)
    i_d = nc.declare_dram_parameter("Ident", [128, 128], f32, isOutput=False)
    o_d = nc.declare_dram_parameter("out", [T_CORE, D], f32, isOutput=True)

    AX = mybir.AxisListType
    OP = mybir.AluOpType

    with ExitStack() as ctx:
        tc = ctx.enter_context(tile.TileContext(nc))
        consts = ctx.enter_context(tc.tile_pool(name="consts", bufs=1))
        hpool = ctx.enter_context(tc.tile_pool(name="h", bufs=2 * N_SUB))
        htpool = ctx.enter_context(tc.tile_pool(name="hT", bufs=4))
        utspool = ctx.enter_context(tc.tile_pool(name="uts", bufs=2))
        outpool = ctx.enter_context(tc.tile_pool(name="osb", bufs=3))
        gpool = ctx.enter_context(tc.tile_pool(name="gates", bufs=2))
        ps_ht = ctx.enter_context(tc.tile_pool(name="ps_ht", bufs=2, space="PSUM"))
        # gT, G, U rotate through one 3-slot pool (1 bank each)
        ps_acc = ctx.enter_context(tc.tile_pool(name="ps_acc", bufs=3, space="PSUM"))
        ps_out = ctx.enter_context(tc.tile_pool(name="ps_out", bufs=3, space="PSUM"))

        A_raw = consts.tile([128, KD, C], f32)
        nc.sync.dma_start(out=A_raw, in_=a_d.rearrange("(k p) c -> p k c", p=128))
        A_sb = consts.tile([128, KD, C], mm_dt)
        nc.vector.tensor_copy(out=A_sb, in_=A_raw)
        B_raw = consts.tile([C, D], f32)
        nc.sync.dma_start(out=B_raw, in_=b_d[:, :])
        B_sb = consts.tile([C, D], mm_dt)
        nc.vector.tensor_copy(out=B_sb, in_=B_raw)
        M_sb = consts.tile([E, C], f32)
        nc.sync.dma_start(out=M_sb, in_=m_d[:, :])
        I_sb = consts.tile([128, 128], f32)
        nc.sync.dma_start(out=I_sb, in_=i_d[:, :])

        for g in range(N_GROUPS):
            t0 = g * GROUP

            h_tiles = []
            for s in range(N_SUB):
                ht = hpool.tile([128, D], f32, tag="h")
                nc.sync.dma_start(
                    out=ht, in_=h_d[t0 + s * 128 : t0 + (s + 1) * 128, :]
                )
                h_tiles.append(ht)

            # ---- top-2 gates on [128 tokens, N_SUB, E] ----
            p_sb = gpool.tile([128, N_SUB, E], f32, tag="p")
            nc.sync.dma_start(
                out=p_sb,
                in_=p_d[t0 : t0 + GROUP, :].rearrange("(s p) e -> p s e", p=128),
            )
            m1 = gpool.tile([128, N_SUB, 1], f32, tag="m1")
            nc.vector.tensor_reduce(out=m1, in_=p_sb, axis=AX.X, op=OP.max)
            mlt = gpool.tile([128, N_SUB, E], f32, tag="mlt")
            nc.vector.tensor_tensor(
                out=mlt, in0=p_sb, in1=m1.broadcast_to([128, N_SUB, E]), op=OP.is_lt
            )
            pm = gpool.tile([128, N_SUB, E], f32, tag="pm")
            nc.vector.tensor_mul(pm, p_sb, mlt)
            m2 = gpool.tile([128, N_SUB, 1], f32, tag="m2")
            nc.vector.tensor_reduce(out=m2, in_=pm, axis=AX.X, op=OP.max)
            ge2 = gpool.tile([128, N_SUB, E], f32, tag="ge2")
            nc.vector.tensor_tensor(
                out=ge2, in0=p_sb, in1=m2.broadcast_to([128, N_SUB, E]), op=OP.is_ge
            )
            gts = gpool.tile([128, N_SUB, E], f32, tag="gts")
            nc.vector.tensor_mul(gts, p_sb, ge2)

            # transpose gates -> gT[e, t] and expand to G[c, t] via one-hot matmul
            gt_ps = ps_acc.tile([128, GROUP], f32, tag="acc")
            for s in range(N_SUB):
                nc.tensor.transpose(
                    out=gt_ps[:E, s * 128 : (s + 1) * 128],
                    in_=gts[:, s, :],
                    identity=I_sb,
                )
            gt_sb = gpool.tile([E, GROUP], f32, tag="gtsb")
            nc.vector.tensor_copy(out=gt_sb, in_=gt_ps[:E, :])
            G_ps = ps_acc.tile([128, GROUP], f32, tag="acc")
            nc.tensor.matmul(G_ps, lhsT=M_sb, rhs=gt_sb, start=True, stop=True)
            G_sb = gpool.tile([128, GROUP], f32, tag="gsb")
            nc.vector.tensor_copy(out=G_sb, in_=G_ps)

            # ---- stage 1: U^T[c, t] accumulated over 16 d-chunks ----
            U_ps = ps_acc.tile([128, GROUP], f32, tag="acc")
            for k in range(KD):
                ht_ps = ps_ht.tile([128, GROUP], f32, tag="htp")
                for s in range(N_SUB):
                    nc.tensor.transpose(
                        out=ht_ps[:, s * 128 : (s + 1) * 128],
                        in_=h_tiles[s][:, k * 128 : (k + 1) * 128],
                        identity=I_sb,
                    )
                ht_sb = htpool.tile([128, GROUP], mm_dt, tag="hts")
                # DVE copies ~1.6x faster than ACT: split 10/6 per group
                if k % 8 < 5:
                    nc.vector.tensor_copy(out=ht_sb, in_=ht_ps)
                else:
                    nc.scalar.copy(out=ht_sb, in_=ht_ps)
                nc.tensor.matmul(
                    U_ps,
                    lhsT=A_sb[:, k, :],
                    rhs=ht_sb[:, :],
                    start=(k == 0),
                    stop=(k == KD - 1),
                )

            # ---- gating ----
            uts = utspool.tile([128, GROUP], mm_dt, tag="uts")
            nc.vector.tensor_tensor(out=uts, in0=U_ps, in1=G_sb, op=OP.mult)

            # ---- stage 2: out[t, d] per 128-token sub-tile ----
            for s in range(N_SUB):
                o_sb = outpool.tile([128, D], f32, tag="osb")
                for j in range(D // 512):
                    o_ps = ps_out.tile([128, 512], f32, tag="ops")
                    nc.tensor.matmul(
                        o_ps,
                        lhsT=uts[:, s * 128 : (s + 1) * 128],
                        rhs=B_sb[:, j * 512 : (j + 1) * 512],
                        start=True,
                        stop=True,
                    )
                    if (s * 4 + j) % 16 < 9:
                        nc.vector.tensor_copy(
                            out=o_sb[:, j * 512 : (j + 1) * 512], in_=o_ps
                        )
                    else:
                        nc.scalar.copy(out=o_sb[:, j * 512 : (j + 1) * 512], in_=o_ps)
                nc.sync.dma_start(
                    out=o_d[t0 + s * 128 : t0 + (s + 1) * 128, :], in_=o_sb
                )

    if split_waits:
        _split_matmul_waits(nc)
    return nc


def _split_matmul_waits(nc, max_waits=1):
    """Walrus codegen allows only one sync-wait slot on self-loading
    (fp32/fp32r) Matmult instructions. Move surplus waits onto a no-op
    EventSemaphore inserted immediately before, same engine — identical
    semantics (waits still complete before the matmul dispatches)."""
    import concourse.mybir as mybir

    n = 0
    for f in nc.m.functions:
        for blk in f.blocks:
            insts = blk.instructions
            new_list = []
            changed = False
            for inst in insts:
                si = inst.sync_info
                if (
                    type(inst).__name__ != "InstEventSemaphore"
                    and si is not None
                    and si.on_wait
                    and len(si.on_wait) > max_waits
                ):
                    surplus = list(si.on_wait[:-max_waits])
                    keep = list(si.on_wait[-max_waits:])
                    # EventSemaphore carriers take at most 2 waits each
                    for i in range(0, len(surplus), 2):
                        n += 1
                        ev = mybir.InstEventSemaphore(
                            name=f"I-swsplit-{n}", ins=[], outs=[]
                        )
                        ev.engine = inst.engine
                        ev.sync_info = mybir.SyncInfo(
                            on_wait=surplus[i : i + 2], on_update=[]
                        )
                        new_list.append(ev)
                    inst.sync_info = mybir.SyncInfo(
                        on_wait=keep, on_update=list(si.on_update or [])
                    )
                    changed = True
                new_list.append(inst)
            if changed:
                blk.instructions = new_list
    return n


def _host_prep(h, p_L, A, B):
    """Shard tokens across cores; build replicated helper matrices."""
    h_flat = np.ascontiguousarray(h.reshape(T_FULL, D), dtype=np.float32)
    p_flat = np.ascontiguousarray(p_L.reshape(T_FULL, E), dtype=np.float32)
    # A_cat[d, c] = SCALING * A[e, r, d]
    A_cat = np.ascontiguousarray(
        (np.asarray(A, dtype=np.float32) * SCALING).transpose(2, 0, 1).reshape(D, C)
    )
    # B_cat[c, d] = B[e, d, r]
    B_cat = np.ascontiguousarray(
        np.asarray(B, dtype=np.float32).transpose(0, 2, 1).reshape(C, D)
    )
    Mexp = np.zeros((E, C), dtype=np.float32)
    for e in range(E):
        Mexp[e, e * R : (e + 1) * R] = 1.0
    Ident = np.eye(128, dtype=np.float32)
    in_maps = []
    for i in range(N_CORES):
        sl = slice(i * T_CORE, (i + 1) * T_CORE)
        in_maps.append(
            {
                "h": h_flat[sl],
                "p_L": p_flat[sl],
                "A_cat": A_cat,
                "B_cat": B_cat,
                "Mexp": Mexp,
                "Ident": Ident,
            }
        )
    return in_maps


def _get_nc():
    if "nc" not in _CACHE:
        _CACHE["nc"] = _build_nc()
    return _CACHE["nc"]


def kernel(h, p_L, A, B):
    from concourse.bass_utils import run_bass_kernel_spmd

    nc = _get_nc()
    in_maps = _host_prep(h, p_L, A, B)
    res = run_bass_kernel_spmd(nc, in_maps, core_ids=list(range(N_CORES)))
    out = np.concatenate([res.results[i]["out"] for i in range(N_CORES)], axis=0)
    return out.reshape(B_SZ, S_SZ, D)



# revision 3
# speedup vs baseline: 1.9372x; 1.9372x over previous
"""Trainium2 Bass kernel for nn_LoRAPool (MoE top-2 LoRA expert pool).

Math (reference):
    gates[t,e] = p_L[t,e] if e in top-2 of p_L[t,:] else 0
    hr[t,e,r]  = sum_d h[t,d] * A[e,r,d]
    out[t,d]   = sum_{e,r} hr[t,e,r] * 2.0 * gates[t,e] * B[e,d,r]

Folded into two dense matmuls over c = (e,r) in [0,128):
    A_cat[d,c] = 2.0 * A[e,r,d];  B_cat[c,d] = B[e,d,r]
    U^T[c,t]   = sum_d A_cat[d,c] hT[d,t]       (stage 1, PE, bf16)
    Us[c,t]    = U^T[c,t] * gates[t, c//16]     (gating, DVE, f32-exact gates)
    out[t,d]   = sum_c Us[c,t] B_cat[c,d]       (stage 2, PE, bf16)

Memory-bound problem: all large traffic (h in, out) is bf16 (tolerance is
2e-2; bf16 end-to-end error is ~5e-3). h is pre-transposed on the host so
no on-device transposes are needed and every DMA line is 4 KB contiguous.

Sharding: tokens (4*4096 = 16384) split evenly across 8 cores; weights and
helper matrices replicated.
"""

import numpy as np

N_CORES = 8
B_SZ, S_SZ, D = 4, 4096, 2048
E, R, C = 8, 16, 128
T_FULL = B_SZ * S_SZ            # 16384 tokens
T_CORE = T_FULL // N_CORES      # 2048 tokens per core
GROUP = 512                     # token group (stage-1 PSUM bank width)
N_GROUPS = T_CORE // GROUP      # 4
N_SUBTOT = T_CORE // 128        # 16 sub-blocks of 128 tokens per core
SUB_PER_GROUP = GROUP // 128    # 4
KD = D // 128                   # 16 contraction chunks
SCALING = 2.0

_CACHE = {}


def _build_nc(split_waits=True):
    import concourse.bass as bass
    import concourse.tile as tile
    import concourse.mybir as mybir
    from contextlib import ExitStack

    f32 = mybir.dt.float32
    f32r = mybir.dt.float32r
    bf16 = mybir.dt.bfloat16

    nc = bass.Bass()
    ht_d = nc.declare_dram_parameter("hT", [D, T_CORE], bf16, isOutput=False)
    p_d = nc.declare_dram_parameter("p_perm", [T_CORE, E], f32, isOutput=False)
    a_d = nc.declare_dram_parameter("A_cat", [128, KD * C], bf16, isOutput=False)
    b_d = nc.declare_dram_parameter("B_cat", [C, D], bf16, isOutput=False)
    m_d = nc.declare_dram_parameter("Mexp", [E, C], f32, isOutput=False)
    i_d = nc.declare_dram_parameter("Ident", [128, 128], f32, isOutput=False)
    o_d = nc.declare_dram_parameter("out", [T_CORE, D], bf16, isOutput=True)

    AX = mybir.AxisListType
    OP = mybir.AluOpType

    with ExitStack() as ctx:
        tc = ctx.enter_context(tile.TileContext(nc))
        consts = ctx.enter_context(tc.tile_pool(name="consts", bufs=1))
        hpool = ctx.enter_context(tc.tile_pool(name="h", bufs=KD))
        gpool = ctx.enter_context(tc.tile_pool(name="gates", bufs=1))
        gtpool = ctx.enter_context(tc.tile_pool(name="gt", bufs=2))
        gsbpool = ctx.enter_context(tc.tile_pool(name="gsb", bufs=N_GROUPS))
        utspool = ctx.enter_context(tc.tile_pool(name="uts", bufs=N_GROUPS))
        opool = ctx.enter_context(tc.tile_pool(name="osb", bufs=3))
        ps_u = ctx.enter_context(tc.tile_pool(name="ps_u", bufs=N_GROUPS, space="PSUM"))
        # gt/G (early) and stage-2 out tiles (late) share one 4-bank pool
        ps_misc = ctx.enter_context(tc.tile_pool(name="ps_misc", bufs=4, space="PSUM"))

        # ---- replicated constants (issue first so they clear the queue) ----
        A_sb = consts.tile([128, KD * C], bf16)
        nc.sync.dma_start(out=A_sb, in_=a_d[:, :])
        B_sb = consts.tile([C, D], bf16)
        nc.sync.dma_start(out=B_sb, in_=b_d[:, :])
        M_raw = consts.tile([E, C], f32)
        nc.sync.dma_start(out=M_raw, in_=m_d[:, :])
        M_sb = consts.tile([E, C], f32r)
        nc.vector.tensor_copy(out=M_sb, in_=M_raw)
        I_sb = consts.tile([128, 128], f32)
        nc.sync.dma_start(out=I_sb, in_=i_d[:, :])

        # ---- top-2 gates for the whole core: [128 tok, 16 sub, 8 exp] ----
        p_sb = gpool.tile([128, N_SUBTOT, E], f32)
        nc.sync.dma_start(out=p_sb, in_=p_d.rearrange("(p n) e -> p n e", p=128))
        m1 = gpool.tile([128, N_SUBTOT, 1], f32)
        nc.vector.tensor_reduce(out=m1, in_=p_sb, axis=AX.X, op=OP.max)
        mlt = gpool.tile([128, N_SUBTOT, E], f32)
        nc.vector.tensor_tensor(
            out=mlt, in0=p_sb, in1=m1.broadcast_to([128, N_SUBTOT, E]), op=OP.is_lt
        )
        pm = gpool.tile([128, N_SUBTOT, E], f32)
        nc.vector.tensor_mul(pm, p_sb, mlt)
        m2 = gpool.tile([128, N_SUBTOT, 1], f32)
        nc.vector.tensor_reduce(out=m2, in_=pm, axis=AX.X, op=OP.max)
        ge2 = gpool.tile([128, N_SUBTOT, E], f32)
        nc.vector.tensor_tensor(
            out=ge2, in0=p_sb, in1=m2.broadcast_to([128, N_SUBTOT, E]), op=OP.is_ge
        )
        gts = gpool.tile([128, N_SUBTOT, E], f32)
        nc.vector.tensor_mul(gts, p_sb, ge2)

        # expand to dense G[c, t] per group via transpose + one-hot matmul
        G_sbs = []
        for g in range(N_GROUPS):
            gt_ps = ps_misc.tile([128, GROUP], f32, tag="misc")
            for s4 in range(SUB_PER_GROUP):
                s = g * SUB_PER_GROUP + s4
                nc.tensor.transpose(
                    out=gt_ps[:E, s4 * 128 : (s4 + 1) * 128],
                    in_=gts[:, s, :],
                    identity=I_sb,
                )
            gt_sb = gtpool.tile([E, GROUP], f32r, tag="gtsb")
            nc.vector.tensor_copy(out=gt_sb, in_=gt_ps[:E, :])
            G_ps = ps_misc.tile([128, GROUP], f32, tag="misc")
            nc.tensor.matmul(G_ps, lhsT=M_sb, rhs=gt_sb, start=True, stop=True)
            G_sb = gsbpool.tile([128, GROUP], f32, tag="gsb")
            nc.scalar.copy(out=G_sb, in_=G_ps)
            G_sbs.append(G_sb)

        # ---- stream h^T chunks; stage 1 accumulates all groups in parallel ----
        ht_tiles = []
        for k in range(KD):
            ht = hpool.tile([128, T_CORE], bf16, tag="h")
            nc.sync.dma_start(out=ht, in_=ht_d[k * 128 : (k + 1) * 128, :])
            ht_tiles.append(ht)

        U_tiles = [
            ps_u.tile([128, GROUP], f32, tag="u", name=f"U{g}")
            for g in range(N_GROUPS)
        ]
        for k in range(KD):
            for g in range(N_GROUPS):
                nc.tensor.matmul(
                    U_tiles[g],
                    lhsT=A_sb[:, k * C : (k + 1) * C],
                    rhs=ht_tiles[k][:, g * GROUP : (g + 1) * GROUP],
                    start=(k == 0),
                    stop=(k == KD - 1),
                )

        # ---- gating + stage 2 + store ----
        copy_flip = 0
        for g in range(N_GROUPS):
            uts = utspool.tile([128, GROUP], bf16, tag="uts")
            nc.vector.tensor_tensor(
                out=uts, in0=U_tiles[g], in1=G_sbs[g], op=OP.mult
            )
            for s4 in range(SUB_PER_GROUP):
                s = g * SUB_PER_GROUP + s4
                o_sb = opool.tile([128, D], bf16, tag="osb")
                for j in range(D // 512):
                    o_ps = ps_misc.tile([128, 512], f32, tag="misc")
                    nc.tensor.matmul(
                        o_ps,
                        lhsT=uts[:, s4 * 128 : (s4 + 1) * 128],
                        rhs=B_sb[:, j * 512 : (j + 1) * 512],
                        start=True,
                        stop=True,
                    )
                    if copy_flip % 2 == 0:
                        nc.vector.tensor_copy(
                            out=o_sb[:, j * 512 : (j + 1) * 512], in_=o_ps
                        )
                    else:
                        nc.scalar.copy(out=o_sb[:, j * 512 : (j + 1) * 512], in_=o_ps)
                    copy_flip += 1
                nc.sync.dma_start(
                    out=o_d[s * 128 : (s + 1) * 128, :], in_=o_sb
                )

    if split_waits:
        _split_matmul_waits(nc)
    return nc


def _split_matmul_waits(nc, max_waits=1):
    """Walrus codegen allows only one sync-wait slot on self-loading
    Matmult instructions. Move surplus waits onto a no-op EventSemaphore
    inserted immediately before, same engine — identical semantics."""
    import concourse.mybir as mybir

    n = 0
    for f in nc.m.functions:
        for blk in f.blocks:
            insts = blk.instructions
            new_list = []
            changed = False
            for inst in insts:
                si = inst.sync_info
                if (
                    type(inst).__name__ != "InstEventSemaphore"
                    and si is not None
                    and si.on_wait
                    and len(si.on_wait) > max_waits
                ):
                    surplus = list(si.on_wait[:-max_waits])
                    keep = list(si.on_wait[-max_waits:])
                    for i in range(0, len(surplus), 2):
                        n += 1
                        ev = mybir.InstEventSemaphore(
                            name=f"I-swsplit-{n}", ins=[], outs=[]
                        )
                        ev.engine = inst.engine
                        ev.sync_info = mybir.SyncInfo(
                            on_wait=surplus[i : i + 2], on_update=[]
                        )
                        new_list.append(ev)
                    inst.sync_info = mybir.SyncInfo(
                        on_wait=keep, on_update=list(si.on_update or [])
                    )
                    changed = True
                new_list.append(inst)
            if changed:
                blk.instructions = new_list
    return n


def _host_prep(h, p_L, A, B):
    """Shard tokens across cores; pre-transpose h; build helper matrices."""
    import ml_dtypes

    BF16 = ml_dtypes.bfloat16

    h3 = np.asarray(h, dtype=np.float32).reshape(N_CORES, T_CORE, D)
    # hT[core][d, t] in bf16 — transposed on host so device DMAs are contiguous
    hT = h3.transpose(0, 2, 1).astype(BF16)
    hT = np.ascontiguousarray(hT)

    # permute p_L rows so partition p holds tokens {n*128+p}: row p*16+n
    p3 = np.asarray(p_L, dtype=np.float32).reshape(
        N_CORES, N_SUBTOT, 128, E
    )
    p_perm = np.ascontiguousarray(p3.transpose(0, 2, 1, 3)).reshape(
        N_CORES, T_CORE, E
    )

    # A_cat[d, c] = SCALING * A[e, r, d], pre-arranged [p, k*C + c]
    A_cat = (np.asarray(A, dtype=np.float32) * SCALING).transpose(2, 0, 1).reshape(D, C)
    A_arr = np.ascontiguousarray(
        A_cat.reshape(KD, 128, C).transpose(1, 0, 2).reshape(128, KD * C)
    ).astype(BF16)
    # B_cat[c, d] = B[e, d, r]
    B_cat = (
        np.asarray(B, dtype=np.float32).transpose(0, 2, 1).reshape(C, D).astype(BF16)
    )
    Mexp = np.zeros((E, C), dtype=np.float32)
    for e in range(E):
        Mexp[e, e * R : (e + 1) * R] = 1.0
    Ident = np.eye(128, dtype=np.float32)

    in_maps = []
    for i in range(N_CORES):
        in_maps.append(
            {
                "hT": hT[i],
                "p_perm": p_perm[i],
                "A_cat": A_arr,
                "B_cat": B_cat,
                "Mexp": Mexp,
                "Ident": Ident,
            }
        )
    return in_maps


def _get_nc():
    if "nc" not in _CACHE:
        _CACHE["nc"] = _build_nc()
    return _CACHE["nc"]


def kernel(h, p_L, A, B):
    from concourse.bass_utils import run_bass_kernel_spmd

    nc = _get_nc()
    in_maps = _host_prep(h, p_L, A, B)
    res = run_bass_kernel_spmd(nc, in_maps, core_ids=list(range(N_CORES)))
    out = np.concatenate(
        [np.asarray(res.results[i]["out"]) for i in range(N_CORES)], axis=0
    )
    return out.astype(np.float32).reshape(B_SZ, S_SZ, D)
